# revision 22
# baseline (speedup 1.0000x reference)
"""Trainium2 Bass kernel for nn_Attention_31705448579931.

Multi-head attention (b=16, L=784, dim=384, H=8, qk=32, v=128) with a
bicubic-resampled relative-position bias:

    out = proj( softmax(q k^T/sqrt(d) + M ab M^T) v )

Sharding: data-parallel over batch — each of the 8 NeuronCores handles 2
batches and all 8 heads.

The relative-position bias depends only on the (weight-like) inputs
ab_table/bias_idxs, not on x, so exp(B^T) is constant-folded on the host
(fp32, more accurate than on-device bf16 interpolation) and streamed per
head from DRAM; the bias enters softmax multiplicatively via
exp(S+B) = exp(S) * exp(B).

Device schedule (driven by trace analysis):
  - attention is computed k-major: S^T tiles (kpos on partitions, q on free)
    so exp(S^T) is directly the lhsT-side operand for the P@V matmul; the two
    N-chunks of each S^T tile run as row-group-packed concurrent K=32 matmuls.
  - the PV/ones accumulation chains of instance i-2 are interleaved four
    matmuls at a time between instance i's S^T pairs: the S^T matmuls are
    exp-rate-limited (each pair waits for a ScalarE exp to free its PSUM
    slot), and the always-ready chain matmuls fill those ~0.5us bubbles so
    the PE queue stays dense and the HAM clock stays un-throttled.
  - PSUM: S^T tiles rotate through a 2-slot ring; the PV and ones chain
    outputs own fixed banks (all fp32) — a shared ring would deadlock the
    interleave (an S tile would wait on a chain queued behind it).
  - softmax denominators via an all-ones (112,128) matmul chain, a
    lane-parallel fast reciprocal on VectorE and a column scale fused into
    the PSUM->SBUF eviction of the attention output.
  - q/k are evicted from PSUM with ONE full-width cast per head-pair (the
    previous four 1x-rate casts serialized the PE), then a partition-swapped
    copy D = swap32(C) gives every head matching q/k partition windows for
    the row-group packing: head even uses C[0:32]/D[0:32] + C[32:64]/D[32:64]
    on row groups 0/1, head odd the same at +64 on row groups 2/3.
  - the output-projection bias is applied by a K=1 matmul appended to the
    projection chain (lhsT = ones column, rhs = bf16 bproj), so the PSUM
    eviction is a plain 2x-rate copy and VectorE stays off the tail's
    critical path; output tiles rotate through 4 buffers so the copy->DMA
    tail pipelines.

All matmuls run in bf16 (inputs pre-cast on host) with fp32 PSUM accumulation.
"""

import numpy as np
import ml_dtypes

import concourse.mybir as mybir
import concourse.tile as tile
from concourse import bacc
from concourse.bass_utils import run_bass_kernel_spmd

N_CORES = 8
B = 16          # global batch
BC = B // N_CORES  # batches per core
L = 784
DIM = 384
H = 8
QK = 32
VD = 128
RES = 25
N = RES * RES   # 625
A_CUBIC = -0.75
SCALE = QK ** -0.5

LT = 7          # l tiles of 112
LTS = 112
F32 = mybir.dt.float32
BF16 = mybir.dt.bfloat16
BF16_NP = ml_dtypes.bfloat16

NSPLITS = [(0, 512), (512, L)]  # free-dim chunks for 784-wide matmul outputs


def _cubic_weight(x):
    ax = np.abs(x)
    a = A_CUBIC
    w1 = ((a + 2.0) * ax - (a + 3.0)) * ax * ax + 1.0
    w2 = a * (((ax - 5.0) * ax + 8.0) * ax - 4.0)
    return np.where(ax <= 1.0, w1, np.where(ax < 2.0, w2, 0.0)).astype(np.float32)


def interp_matrix(Lo, Li):
    """Dense 1-D bicubic resampling matrix (Lo, Li), matches reference."""
    scale = Li / Lo
    src = (np.arange(Lo, dtype=np.float32) + 0.5) * scale - 0.5
    f = np.floor(src)
    t = (src - f).astype(np.float32)
    ws = np.stack(
        [_cubic_weight(t + 1.0), _cubic_weight(t), _cubic_weight(1.0 - t),
         _cubic_weight(2.0 - t)], axis=1)
    idx = f.astype(np.int32)[:, None] + np.arange(-1, 3, dtype=np.int32)[None, :]
    idx = np.clip(idx, 0, Li - 1)
    M = np.zeros((Lo, Li), dtype=np.float32)
    np.add.at(M, (np.arange(Lo)[:, None], idx), ws)
    return M


_BUILD_CACHE = {}


def build():
    if "nc" in _BUILD_CACHE:
        return _BUILD_CACHE["nc"]

    nc = bacc.Bacc("TRN2", target_bir_lowering=False, debug=False,
                   num_devices=N_CORES)

    xT_e = nc.dram_tensor("xT", [BC, DIM, L], BF16, kind="ExternalInput")
    wqkvT_e = nc.dram_tensor("wqkvT", [DIM, 1536], BF16, kind="ExternalInput")
    wprojT_e = nc.dram_tensor("wprojT", [H * VD, DIM], BF16, kind="ExternalInput")
    bproj_e = nc.dram_tensor("bproj", [1, DIM], BF16, kind="ExternalInput")
    eb_e = nc.dram_tensor("eb", [H * L, L], BF16, kind="ExternalInput")
    out_e = nc.dram_tensor("out", [BC, L, DIM], F32, kind="ExternalOutput")

    with tile.TileContext(nc) as tc:
        with (
            tc.tile_pool(name="const", bufs=1) as constp,
            tc.tile_pool(name="wq", bufs=1) as wqp,
            tc.tile_pool(name="x", bufs=1) as xp,
            tc.tile_pool(name="v", bufs=1) as vp,
            tc.tile_pool(name="qk", bufs=1) as qkp,
            tc.tile_pool(name="bias", bufs=2) as biasp,
            tc.tile_pool(name="attn", bufs=4) as attnp,
            tc.tile_pool(name="mis", bufs=1) as misp,
            tc.tile_pool(name="ot", bufs=1) as otp,
            tc.tile_pool(name="ps", bufs=4, space="PSUM") as psp,
        ):
            # ---- input DMAs ----------------------------------------------
            # scalar queue: qkv weights (gate V/qk projections)
            wq_sb = []
            for kc in range(3):
                t = wqp.tile([128, 1536], BF16, tag=f"wq{kc}")
                nc.scalar.dma_start(t[:], wqkvT_e[kc * 128:(kc + 1) * 128, :])
                wq_sb.append(t)
            # sync queue: x, bias row, then proj weights
            bproj_row = constp.tile([1, DIM], BF16, tag="bprow")
            nc.sync.dma_start(bproj_row[:], bproj_e[:, :])
            xT_sb = [[None] * 3 for _ in range(BC)]
            for b in range(BC):
                for kc in range(3):
                    t = xp.tile([128, L], BF16, tag=f"x{b}{kc}")
                    nc.sync.dma_start(t[:], xT_e[b, kc * 128:(kc + 1) * 128, :])
                    xT_sb[b][kc] = t
            wproj_sb = []
            for h in range(H):
                t = wqp.tile([128, DIM], BF16, tag=f"wp{h}")
                nc.sync.dma_start(t[:], wprojT_e[h * VD:(h + 1) * VD, :])
                wproj_sb.append(t)

            # preload the exp activation table (~2.7us) during the DMA wait
            dummy_in = constp.tile([1, 8], F32, tag="dummy_in")
            nc.vector.memset(dummy_in[:], 0.0)
            dummy_out = constp.tile([1, 8], F32, tag="dummy_out")
            nc.scalar.activation(dummy_out[:], dummy_in[:],
                                 mybir.ActivationFunctionType.Exp)

            ones_l = constp.tile([LTS, VD], BF16, tag="ones")
            nc.any.memset(ones_l[:], 1.0)
            ones_col = constp.tile([1, LTS], BF16, tag="onesc")
            nc.any.memset(ones_col[:], 1.0)

            # ---- PE warmup on memset tiles: covers the input-DMA wait and
            # releases the HAM clock throttle (1.2 -> 2.4 GHz).
            warm_rhs = constp.tile([LTS, 512], BF16, tag="warmr")
            nc.any.memset(warm_rhs[:], 0.5)
            warm_ps = psp.tile([VD, 512], F32, tag="ps")
            for wi in range(8):
                nc.tensor.matmul(
                    warm_ps[:],
                    lhsT=ones_l[:],
                    rhs=warm_rhs[:],
                    start=(wi == 0), stop=(wi == 7),
                )

            # ---- V projection (l-major: kpos on partitions) ---------------
            v_sb = [[None] * LT for _ in range(BC)]
            for b in range(BC):
                for lt in range(LT):
                    pv = psp.tile([LTS, 1024], F32, tag="ps")
                    for half in range(2):
                        for kc in range(3):
                            nc.tensor.matmul(
                                pv[:, half * 512:(half + 1) * 512],
                                lhsT=xT_sb[b][kc][:, lt * LTS:(lt + 1) * LTS],
                                rhs=wq_sb[kc][:, 512 + half * 512:1024 + half * 512],
                                start=(kc == 0), stop=(kc == 2),
                            )
                    vt = vp.tile([LTS, 1024], BF16, tag=f"v{b}{lt}")
                    nc.scalar.copy(vt[:], pv[:])
                    v_sb[b][lt] = vt

            # ---- q/k projections for all heads ----------------------------
            # pqk rows are [q_h0, k_h0, q_h1, k_h1] (32 each); one full-width
            # cast C, then a 32-partition swap D so each head sees q and k in
            # the SAME two partition windows for row-group packing.
            qk_all = {}
            for mt_i in range(H // 2):
                for b in range(BC):
                    pqk = psp.tile([128, L], F32, tag="ps")
                    for (ns, ne) in NSPLITS:
                        for kc in range(3):
                            nc.tensor.matmul(
                                pqk[:, ns:ne],
                                lhsT=wq_sb[kc][:, mt_i * 128:(mt_i + 1) * 128],
                                rhs=xT_sb[b][kc][:, ns:ne],
                                start=(kc == 0), stop=(kc == 2),
                            )
                    cq = qkp.tile([128, L], BF16, tag=f"C{b}{mt_i}")
                    nc.vector.tensor_copy(cq[:], pqk[:])
                    dq = qkp.tile([128, L], BF16, tag=f"D{b}{mt_i}")
                    nc.sync.dma_start(dq[0:32, :], cq[32:64, :])
                    nc.sync.dma_start(dq[32:64, :], cq[0:32, :])
                    nc.sync.dma_start(dq[64:96, :], cq[96:128, :])
                    nc.sync.dma_start(dq[96:128, :], cq[64:96, :])
                    # (C, D, off): q rows at C[off:off+32] / D[off+32:off+64],
                    # k rows at D[off:off+32] / C[off+32:off+64]
                    qk_all[(b, 2 * mt_i)] = (cq, dq, 0)
                    qk_all[(b, 2 * mt_i + 1)] = (cq, dq, 64)

            # ---- per-head loop --------------------------------------------
            ot_sb = [[None] * H for _ in range(BC)]
            _pend = []

            def _flush_pv(item):
                """Emit the ones/PV accumulation chains for a finished
                instance as uninterrupted same-bank chains (they pipeline on
                the PE).  Deliberately NOT interleaved into the S^T stream:
                a perfectly dense PE stream trips the firmware power limiter
                (measured: 208us straight at K=4/8 half-clock), while the
                exp-paced micro-gaps of this schedule keep the average PE
                clock at ~0.83 of nominal."""
                fh, fb, ptiles = item
                ps_one = psp.tile([VD, L], F32, tag="ps")
                for (ns, ne) in NSPLITS:
                    for kt in range(LT):
                        nc.tensor.matmul(
                            ps_one[:, ns:ne],
                            lhsT=ones_l[:],
                            rhs=ptiles[kt][:, ns:ne],
                            start=(kt == 0), stop=(kt == LT - 1),
                        )
                rdb = misp.tile([VD, L], F32, tag=f"rdb{(fh * BC + fb) % 2}")
                nc.vector.reciprocal_approx_fast(rdb[:], ps_one[:])
                ps_o = psp.tile([VD, L], F32, tag="ps")
                for (ns, ne) in NSPLITS:
                    for kt in range(LT):
                        nc.tensor.matmul(
                            ps_o[:, ns:ne],
                            lhsT=v_sb[fb][kt][:, fh * VD:(fh + 1) * VD],
                            rhs=ptiles[kt][:, ns:ne],
                            start=(kt == 0), stop=(kt == LT - 1),
                        )
                ot = otp.tile([VD, L], BF16, tag=f"ot{fb}{fh}")
                nc.vector.tensor_mul(ot[:], ps_o[:], rdb[:])
                ot_sb[fb][fh] = ot

            for h in range(H):
                # --- exp(B^T) tiles for head h: streamed on the (otherwise
                # idle) sync queue, double-buffered across heads ------------
                expb_sb = []
                for pt in range(LT):
                    eb = biasp.tile([LTS, L], BF16, tag=f"eb{pt}")
                    nc.sync.dma_start(
                        eb[:],
                        eb_e[h * L + pt * LTS:h * L + (pt + 1) * LTS, :])
                    expb_sb.append(eb)

                # --- attention for (h, b): emit instance i's S^T/exp/mul with
                # instance i-2's PV/ones chains interleaved 4 matmuls per
                # S^T pair ---------------------------------------------------
                for b in range(BC):
                    if len(_pend) > 1:
                        _flush_pv(_pend.pop(0))
                    qt_c, qt_d, off = qk_all[(b, h)]
                    pt_tiles = []
                    for kt in range(LT):
                        # two BALANCED 392-wide N-chunks as row-group-packed
                        # concurrent K=32 matmuls, one per PSUM bank of a
                        # [112, 2, 512] tile (concurrent row-group outputs
                        # must land in different banks; 392/392 beats the
                        # bank-aligned 512/272 split by 120 cycles of span)
                        ps_s = psp.tile([LTS, 2, 512], F32, tag="ps")
                        nc.tensor.matmul(
                            ps_s[:, 0, 0:392],
                            lhsT=qt_d[off:off + QK, kt * LTS:(kt + 1) * LTS],
                            rhs=qt_c[off:off + QK, 0:392],
                            start=True, stop=True,
                            tile_position=(off, 0),
                        )
                        nc.tensor.matmul(
                            ps_s[:, 1, 0:392],
                            lhsT=qt_c[off + QK:off + 2 * QK, kt * LTS:(kt + 1) * LTS],
                            rhs=qt_d[off + QK:off + 2 * QK, 392:L],
                            start=True, stop=True,
                            tile_position=(off + QK, 0),
                        )
                        ptile = attnp.tile([LTS, L], BF16, tag=f"pT{kt}")
                        nc.scalar.activation(ptile[:], ps_s[:, :, 0:392],
                                             mybir.ActivationFunctionType.Exp)
                        nc.vector.tensor_mul(ptile[:], ptile[:], expb_sb[kt][:])
                        pt_tiles.append(ptile)
                    _pend.append((h, b, pt_tiles))

            # ---- output projection: the bias lands via a K=1 matmul
            # appended to the accumulation chain, so the eviction is a plain
            # 2x-rate copy ---------------------------------------------------
            def _proj(b):
                for lt in range(LT):
                    py = psp.tile([LTS, DIM], F32, tag="ps")
                    for h in range(H):
                        nc.tensor.matmul(
                            py[:],
                            lhsT=ot_sb[b][h][:, lt * LTS:(lt + 1) * LTS],
                            rhs=wproj_sb[h][:],
                            start=(h == 0), stop=False,
                        )
                    nc.tensor.matmul(
                        py[:],
                        lhsT=ones_col[:, :],
                        rhs=bproj_row[:, :],
                        start=False, stop=True,
                    )
                    ysb = misp.tile([LTS, DIM], F32, tag=f"y{lt % 4}")
                    nc.vector.tensor_copy(ysb[:], py[:])
                    nc.sync.dma_start(
                        out_e[b, lt * LTS:(lt + 1) * LTS, :], ysb[:])

            while _pend:
                fh, fb, ptiles = _pend[0]
                _flush_pv(_pend.pop(0))
                if fh == H - 1:
                    _proj(fb)

    nc.compile()
    _BUILD_CACHE["nc"] = nc
    return nc


def _prep_in_maps(inputs):
    x = np.asarray(inputs["x"], dtype=np.float32)
    Wqkv = np.asarray(inputs["Wqkv"], dtype=np.float32)
    Wproj = np.asarray(inputs["Wproj"], dtype=np.float32)
    bproj = np.asarray(inputs["bproj"], dtype=np.float32)
    ab_table = np.asarray(inputs["ab_table"], dtype=np.float32)
    bias_idxs = np.asarray(inputs["bias_idxs"])

    # reorder qkv weight rows: [q0 k0 q1 k1 ... q7 k7 | v0 v1 ... v7]
    w3 = Wqkv.reshape(H, 2 * QK + VD, DIM)
    qk_rows = np.concatenate(
        [np.concatenate([w3[h, :QK] * SCALE, w3[h, QK:2 * QK]], axis=0)
         for h in range(H)],
        axis=0)                     # (512, 384)
    v_rows = np.concatenate([w3[h, 2 * QK:] for h in range(H)], axis=0)  # (1024,384)
    wqkvT = np.ascontiguousarray(
        np.concatenate([qk_rows, v_rows], axis=0).T).astype(BF16_NP)  # (384,1536)

    wprojT = np.ascontiguousarray(Wproj.T).astype(BF16_NP)  # (1024, 384)
    bproj2 = np.ascontiguousarray(bproj.reshape(1, DIM)).astype(BF16_NP)

    # constant-fold the relative-position bias (input-only, x-independent):
    # B_h = M ab_h M^T; ship exp(B_h^T) = exp(B_h)^T (B symmetric) in bf16.
    M = interp_matrix(L, N)                      # (784, 625)
    ab_all = ab_table[:, bias_idxs]              # (H, 625, 625)
    MT = np.ascontiguousarray(M.T)
    ebias = np.empty((H * L, L), dtype=BF16_NP)
    for h in range(H):
        Bh = (M @ ab_all[h]) @ MT                # (784, 784)
        ebias[h * L:(h + 1) * L] = np.exp(Bh.astype(np.float64)).astype(BF16_NP)

    in_maps = []
    for c in range(N_CORES):
        xT = np.ascontiguousarray(
            x[c * BC:(c + 1) * BC].transpose(0, 2, 1)).astype(BF16_NP)
        in_maps.append({
            "xT": xT,
            "wqkvT": wqkvT,
            "wprojT": wprojT,
            "bproj": bproj2,
            "eb": ebias,
        })
    return in_maps


def _run(inputs, trace=False, **kw):
    nc = build()
    in_maps = _prep_in_maps(inputs)
    last_err = None
    for attempt in range(3):
        try:
            res = run_bass_kernel_spmd(nc, in_maps, core_ids=list(range(N_CORES)),
                                       trace=trace, **kw)
            break
        except Exception as e:  # transient NRT device errors: retry
            last_err = e
    else:
        raise last_err
    out = np.concatenate([res.results[c]["out"] for c in range(N_CORES)], axis=0)
    return out, res


def kernel(**inputs) -> np.ndarray:
    out, _ = _run(inputs, trace=False)
    return out
